# revision 7
# baseline (speedup 1.0000x reference)
"""Trainium2 Bass kernel for the LoRA-mixture layer.

Math (derived from the reference's interleave):  for batch b,
  y[b] = relu( 0.25 * x[b] @ Bcat_b @ Acat_b )
where Bcat_b = concat of adapter_b[4b:4b+4] along rank (rank 16),
      Acat_b = concat of adapter_a[4b:4b+4] along rank.

Sharding: data-parallel, batch b -> core b (8 batches, 8 cores).

The kernel is HBM-bandwidth bound (x in + y out dominate), so all
device I/O is bf16 (rel-err budget 2e-2 >> bf16's ~4e-3). The host
pre-transposes x[b] to xT [D, S] and packs both xT and y so every DMA
is contiguous per partition.

HAM note: the PE clock stays at 1.2 GHz unless the activity monitor
sees matmul work in every ~3.4us window. The input DMA is therefore
split into 0.5 MB quarter-slab transfers with 4 mm1 matmuls attached
to each, which smears PE activity across the whole DMA-bound timeline.

Per-core dataflow (slab = 512 s-rows, 8 slabs):
  4x: DMA in quarter xt slab [128p, 4c, 512s] bf16 (0.5 MB)
      mm1: hT4[128, 512] += bcat4[128,128].T @ xtChunk[128,512] (4 chunks)
      bcat4 has Bcat replicated at column offsets 0/32/64/96 so hT
      lands replicated at partition offsets 0/32/64/96.
  ACT-evict hT4 -> SBUF bf16
  mm2 (pipelined one slab behind mm1): per s-subtile t, 4 concurrent
      row-group matmuls (tile_position):
      y[128,512] = hT[16,128].T @ Acat[16,512]  (0.25 folded into Acat)
  relu-evict PSUM -> SBUF bf16 (split DVE / ACT)
  DMA out y tile [128p, 2048d] bf16 (0.5 MB) per s-subtile
"""

import numpy as np
import ml_dtypes

import concourse.bass as bass
import concourse.mybir as mybir
import concourse.tile as tile
from concourse import bacc
from concourse.bass_utils import run_bass_kernel_spmd

B, S, D = 8, 4096, 2048
R = 16               # concatenated rank per batch (4 adapters x rank 4)
N_CORES = 8
SLAB = 512           # s rows per slab
NSLAB = S // SLAB    # 8
TS = SLAB // 128     # 4 s-subtiles per slab
DC = D // 128        # 16 contraction chunks
NSUB = 4             # input sub-DMAs per slab
CPS = DC // NSUB     # 4 contraction chunks per sub-DMA
NDP = D // 512       # 4 output-column chunks

BF16 = mybir.dt.bfloat16
F32 = mybir.dt.float32
NPBF16 = ml_dtypes.bfloat16
RELU = mybir.ActivationFunctionType.Relu


def build_nc():
    nc = bacc.Bacc("TRN2", target_bir_lowering=False, debug=False)

    # xt: x[b].T packed as [sl, i, p, cc, s'] with d = (i*CPS+cc)*128 + p,
    # s = sl*512 + s'  -> each (sl, i) sub-DMA is 4 KB contiguous/partition
    xt = nc.dram_tensor(
        "xt", [NSLAB, NSUB, 128, CPS, SLAB], BF16, kind="ExternalInput"
    )
    # bcat4 packed p-major on host: [p, c, r] with d = c*128 + p. Bcat
    # columns replicated at offsets 0/32/64/96 (zeros elsewhere) so mm1
    # emits hT at 4 partition offsets for row-packed mm2.
    bcat4 = nc.dram_tensor("bcat4", [128, DC, 128], BF16, kind="ExternalInput")
    acat = nc.dram_tensor("acat", [R, D], BF16, kind="ExternalInput")
    # y packed as [sl, t, p, d] with s = sl*512 + t*128 + p (plain reshape)
    y = nc.dram_tensor("y", [NSLAB, TS, 128, D], BF16, kind="ExternalOutput")

    with tile.TileContext(nc) as tc:
        with (
            tc.tile_pool(name="const", bufs=1) as cpool,
            tc.tile_pool(name="xin", bufs=10) as xin_pool,
            tc.tile_pool(name="ht", bufs=3) as ht_pool,
            tc.tile_pool(name="yout", bufs=6) as y_pool,
            tc.tile_pool(name="ph", bufs=2, space="PSUM") as ph_pool,
            tc.tile_pool(name="py", bufs=4, space="PSUM") as py_pool,
            tc.tile_pool(name="ptk", bufs=1, space="PSUM") as ptk_pool,
        ):
            # First x sub-DMAs go out before anything else so the SP ring
            # starts streaming immediately; consts ride the ACT ring.
            x_sbs0 = []
            for i in range(NSUB):
                x_sb = xin_pool.tile([128, CPS, SLAB], BF16, tag="xin")
                nc.sync.dma_start(out=x_sb[:], in_=xt.ap()[0, i])
                x_sbs0.append(x_sb)

            bcat_sb = cpool.tile([128, DC, 128], BF16)
            nc.scalar.dma_start(out=bcat_sb[:], in_=bcat4.ap())
            # Acat replicated at partition offsets 0/32/64/96 for row-packed
            # mm2 (rhs partitions must match the row group). Unwritten rows
            # are never read.
            acat_rep = cpool.tile([128, D], BF16)
            for j in range(4):
                nc.scalar.dma_start(
                    out=acat_rep[32 * j : 32 * j + R, :], in_=acat.ap()
                )

            # HAM tickle target: a tiny matmul on const data keeps the PE
            # activity monitor from re-throttling the clock during DMA
            # stalls. The scratch PSUM tile is never read.
            ptick = ptk_pool.tile([128, 64], F32)

            def tickle():
                nc.tensor.matmul(
                    ptick[:], bcat_sb[:, 0, :], bcat_sb[:, 0, :64],
                    start=True, stop=True,
                )

            ht_reps = [None] * NSLAB

            def emit_mm2(k):
                # mm2 for slab k: per s-subtile t, 4 concurrent row-group
                # matmuls (row group j = d'-chunk), relu-evict, DMA out.
                for t in range(TS):
                    tickle()
                    y_sb = y_pool.tile([128, D], BF16, tag="yout")
                    pys = []
                    for j in range(NDP):
                        py = py_pool.tile([128, 512], F32, tag="py")
                        nc.tensor.matmul(
                            py[:],
                            ht_reps[k][32 * j : 32 * j + R, t * 128 : (t + 1) * 128],
                            acat_rep[32 * j : 32 * j + R, j * 512 : (j + 1) * 512],
                            start=True,
                            stop=True,
                            tile_position=(32 * j, 0),
                        )
                        pys.append(py)
                    for j in range(NDP):
                        dst = y_sb[:, j * 512 : (j + 1) * 512]
                        if j < 2:
                            nc.vector.tensor_scalar_max(dst, pys[j][:], 0.0)
                        else:
                            nc.scalar.activation(dst, pys[j][:], RELU)
                    # Alternate the two spare DMA paths for the out stream
                    # (SWDGE/gpsimd and the ACT HWDGE ring) so the output
                    # never serializes on a single queue.
                    eng = nc.gpsimd if t % 2 == 0 else nc.scalar
                    eng.dma_start(out=y.ap()[k, t], in_=y_sb[:])

            for sl in range(NSLAB):
                ht_ps = ph_pool.tile([128, SLAB], F32, tag="ph")
                for i in range(NSUB):
                    if sl == 0:
                        x_sb = x_sbs0[i]
                    else:
                        x_sb = xin_pool.tile([128, CPS, SLAB], BF16, tag="xin")
                        nc.sync.dma_start(out=x_sb[:], in_=xt.ap()[sl, i])
                    tickle()
                    for cc in range(CPS):
                        nc.tensor.matmul(
                            ht_ps[:],
                            bcat_sb[:, i * CPS + cc, :],
                            x_sb[:, cc, :],
                            start=(i == 0 and cc == 0),
                            stop=(i == NSUB - 1 and cc == CPS - 1),
                        )
                ht_rep = ht_pool.tile([128, SLAB], BF16, tag="ht")
                nc.scalar.copy(ht_rep[:], ht_ps[:])
                ht_reps[sl] = ht_rep
                # mm2 lags one slab behind mm1 so the PE never waits on the
                # ACT eviction of hT.
                if sl >= 1:
                    emit_mm2(sl - 1)
            emit_mm2(NSLAB - 1)

    nc.compile()
    return nc


_NC = None


def _get_nc():
    global _NC
    if _NC is None:
        _NC = build_nc()
    return _NC


def make_in_maps(x, adapter_b, adapter_a):
    in_maps = []
    for b in range(B):
        # xT [D, S] -> [sl, i, p, cc, s'], d = (i*CPS+cc)*128+p, s = sl*512+s'
        xt = np.ascontiguousarray(
            x[b].T.reshape(NSUB, CPS, 128, NSLAB, SLAB)
            .transpose(3, 0, 2, 1, 4)
            .astype(NPBF16)
        )
        bc = np.ascontiguousarray(
            adapter_b[4 * b : 4 * b + 4].transpose(1, 0, 2).reshape(D, R)
        ).astype(np.float32)
        bc4 = np.zeros((D, 128), dtype=np.float32)
        for j in range(4):
            bc4[:, 32 * j : 32 * j + R] = bc
        # pack p-major: [D, 128] -> [p, c, r] with d = c*128 + p
        bc4 = np.ascontiguousarray(
            bc4.reshape(DC, 128, 128).transpose(1, 0, 2)
        )
        ac = np.ascontiguousarray(
            adapter_a[4 * b : 4 * b + 4].reshape(R, D) * 0.25
        ).astype(np.float32)
        in_maps.append(
            {
                "xt": xt,
                "bcat4": bc4.astype(NPBF16),
                "acat": ac.astype(NPBF16),
            }
        )
    return in_maps


def run(x, adapter_b, adapter_a, **run_kwargs):
    nc = _get_nc()
    in_maps = make_in_maps(x, adapter_b, adapter_a)
    res = run_bass_kernel_spmd(nc, in_maps, list(range(N_CORES)), **run_kwargs)
    # y packed [sl, t, p, d] -> [s, d]: (sl, t, p) is lexicographic in s
    out = np.stack(
        [
            res.results[i]["y"].reshape(S, D).astype(np.float32)
            for i in range(N_CORES)
        ]
    )
    return out, res


def kernel(x, adapter_b, adapter_a):
    out, _ = run(x, adapter_b, adapter_a)
    return out


# revision 12
# speedup vs baseline: 1.1442x; 1.1442x over previous
"""Trainium2 Bass kernel for the LoRA-mixture layer.

Math (derived from the reference's interleave):  for batch b,
  y[b] = relu( 0.25 * x[b] @ Bcat_b @ Acat_b )
where Bcat_b = concat of adapter_b[4b:4b+4] along rank (rank 16),
      Acat_b = concat of adapter_a[4b:4b+4] along rank.

Sharding: data-parallel, batch b -> core b (8 batches, 8 cores).

The kernel is HBM-bandwidth bound (x in + y out dominate), so all
device I/O is bf16 (rel-err budget 2e-2 >> bf16's ~4e-3). The host
pre-transposes x[b] to xT [D, S] and packs both xT and y so every DMA
is contiguous per partition.

HAM note: the PE clock stays at 1.2 GHz unless the activity monitor
sees matmul work in every ~3.4us window. The input DMA is therefore
split into 0.5 MB quarter-slab transfers with 4 mm1 matmuls attached
to each, which smears PE activity across the whole DMA-bound timeline.

Per-core dataflow (slab = 512 s-rows, 8 slabs):
  4x: DMA in quarter xt slab [128p, 4c, 512s] bf16 (0.5 MB)
      mm1: hT4[128, 512] += bcat4[128,128].T @ xtChunk[128,512] (4 chunks)
      bcat4 has Bcat replicated at column offsets 0/32/64/96 so hT
      lands replicated at partition offsets 0/32/64/96.
  ACT-evict hT4 -> SBUF bf16
  mm2 (pipelined one slab behind mm1): per s-subtile t, 4 concurrent
      row-group matmuls (tile_position):
      y[128,512] = hT[16,128].T @ Acat[16,512]  (0.25 folded into Acat)
  relu-evict PSUM -> SBUF bf16 (split DVE / ACT)
  DMA out y tile [128p, 2048d] bf16 (0.5 MB) per s-subtile
"""

import numpy as np
import ml_dtypes

import concourse.bass as bass
import concourse.mybir as mybir
import concourse.tile as tile
from concourse import bacc
from concourse.bass_utils import run_bass_kernel_spmd

B, S, D = 8, 4096, 2048
R = 16               # concatenated rank per batch (4 adapters x rank 4)
N_CORES = 8
SLAB = 512           # s rows per slab
NSLAB = S // SLAB    # 8
TS = SLAB // 128     # 4 s-subtiles per slab
DC = D // 128        # 16 contraction chunks
NSUB = 4             # input sub-DMAs per slab
CPS = DC // NSUB     # 4 contraction chunks per sub-DMA
NDP = D // 512       # 4 output-column chunks

BF16 = mybir.dt.bfloat16
F32 = mybir.dt.float32
NPBF16 = ml_dtypes.bfloat16
RELU = mybir.ActivationFunctionType.Relu


def build_nc():
    nc = bacc.Bacc("TRN2", target_bir_lowering=False, debug=False)

    # xt: x[b].T packed as [sl, i, p, cc, s'] with d = (i*CPS+cc)*128 + p,
    # s = sl*512 + s'  -> each (sl, i) sub-DMA is 4 KB contiguous/partition
    xt = nc.dram_tensor(
        "xt", [NSLAB, NSUB, 128, CPS, SLAB], BF16, kind="ExternalInput"
    )
    # bcat4 packed p-major on host: [p, c, r] with d = c*128 + p. Bcat
    # columns replicated at offsets 0/32/64/96 (zeros elsewhere) so mm1
    # emits hT at 4 partition offsets for row-packed mm2.
    bcat4 = nc.dram_tensor("bcat4", [128, DC, 128], BF16, kind="ExternalInput")
    # acat4 [128, D]: acat rows replicated at partition offsets 0/32/64/96
    # (zeros elsewhere) -> single full-width DMA
    acat4 = nc.dram_tensor("acat4", [128, D], BF16, kind="ExternalInput")
    # y packed as [sl, t, p, d] with s = sl*512 + t*128 + p (plain reshape)
    y = nc.dram_tensor("y", [NSLAB, TS, 128, D], BF16, kind="ExternalOutput")

    with tile.TileContext(nc) as tc:
        with (
            tc.tile_pool(name="const", bufs=1) as cpool,
            tc.tile_pool(name="xin", bufs=6) as xin_pool,
            tc.tile_pool(name="ht", bufs=3) as ht_pool,
            tc.tile_pool(name="yout", bufs=6) as y_pool,
            tc.tile_pool(name="ph", bufs=2, space="PSUM") as ph_pool,
            tc.tile_pool(name="py", bufs=4, space="PSUM") as py_pool,
            tc.tile_pool(name="ptk", bufs=1, space="PSUM") as ptk_pool,
        ):
            # First x sub-DMAs go out before anything else so the SP ring
            # starts streaming immediately; consts ride the ACT ring.
            x_sbs0 = []
            for i in range(NSUB):
                x_sb = xin_pool.tile([128, CPS, SLAB], BF16, tag="xin")
                nc.sync.dma_start(out=x_sb[:], in_=xt.ap()[0, i])
                x_sbs0.append(x_sb)

            bcat_sb = cpool.tile([128, DC, 128], BF16)
            nc.scalar.dma_start(out=bcat_sb[:], in_=bcat4.ap())
            # Acat replicated at partition offsets 0/32/64/96 for row-packed
            # mm2 (rhs partitions must match the row group). Unwritten rows
            # are never read.
            acat_rep = cpool.tile([128, D], BF16)
            nc.scalar.dma_start(out=acat_rep[:], in_=acat4.ap())

            # HAM tickle target: a tiny matmul on const data keeps the PE
            # activity monitor from re-throttling the clock during DMA
            # stalls. The scratch PSUM tile is never read.
            ptick = ptk_pool.tile([128, 64], F32)

            def tickle():
                nc.tensor.matmul(
                    ptick[:], bcat_sb[:, 0, :], bcat_sb[:, 0, :64],
                    start=True, stop=True,
                )

            ht_reps = [None] * NSLAB

            def emit_mm2(k):
                # mm2 for slab k: per s-subtile t, 4 concurrent row-group
                # matmuls (row group j = d'-chunk), relu-evict, DMA out.
                for t in range(TS):
                    tickle()
                    y_sb = y_pool.tile([128, D], BF16, tag="yout")
                    pys = []
                    for j in range(NDP):
                        py = py_pool.tile([128, 512], F32, tag="py")
                        nc.tensor.matmul(
                            py[:],
                            ht_reps[k][32 * j : 32 * j + R, t * 128 : (t + 1) * 128],
                            acat_rep[32 * j : 32 * j + R, j * 512 : (j + 1) * 512],
                            start=True,
                            stop=True,
                            tile_position=(32 * j, 0),
                        )
                        pys.append(py)
                    for j in range(NDP):
                        dst = y_sb[:, j * 512 : (j + 1) * 512]
                        if j < 2:
                            nc.vector.tensor_scalar_max(dst, pys[j][:], 0.0)
                        else:
                            nc.scalar.activation(dst, pys[j][:], RELU)
                    nc.gpsimd.dma_start(out=y.ap()[k, t], in_=y_sb[:])

            for sl in range(NSLAB):
                ht_ps = ph_pool.tile([128, SLAB], F32, tag="ph")
                for i in range(NSUB):
                    if sl == 0:
                        x_sb = x_sbs0[i]
                    else:
                        x_sb = xin_pool.tile([128, CPS, SLAB], BF16, tag="xin")
                        nc.sync.dma_start(out=x_sb[:], in_=xt.ap()[sl, i])
                    tickle()
                    for cc in range(CPS):
                        nc.tensor.matmul(
                            ht_ps[:],
                            bcat_sb[:, i * CPS + cc, :],
                            x_sb[:, cc, :],
                            start=(i == 0 and cc == 0),
                            stop=(i == NSUB - 1 and cc == CPS - 1),
                        )
                ht_rep = ht_pool.tile([128, SLAB], BF16, tag="ht")
                nc.scalar.copy(ht_rep[:], ht_ps[:])
                ht_reps[sl] = ht_rep
                # mm2 lags one slab behind mm1 so the PE never waits on the
                # ACT eviction of hT.
                if sl >= 1:
                    emit_mm2(sl - 1)
            emit_mm2(NSLAB - 1)

    nc.compile()
    return nc


_NC = None


def _get_nc():
    global _NC
    if _NC is None:
        _NC = build_nc()
    return _NC


def make_in_maps(x, adapter_b, adapter_a):
    in_maps = []
    for b in range(B):
        # xT [D, S] -> [sl, i, p, cc, s'], d = (i*CPS+cc)*128+p, s = sl*512+s'
        xt = np.ascontiguousarray(
            x[b].T.reshape(NSUB, CPS, 128, NSLAB, SLAB)
            .transpose(3, 0, 2, 1, 4)
            .astype(NPBF16)
        )
        bc = np.ascontiguousarray(
            adapter_b[4 * b : 4 * b + 4].transpose(1, 0, 2).reshape(D, R)
        ).astype(np.float32)
        bc4 = np.zeros((D, 128), dtype=np.float32)
        for j in range(4):
            bc4[:, 32 * j : 32 * j + R] = bc
        # pack p-major: [D, 128] -> [p, c, r] with d = c*128 + p
        bc4 = np.ascontiguousarray(
            bc4.reshape(DC, 128, 128).transpose(1, 0, 2)
        )
        ac = np.ascontiguousarray(
            adapter_a[4 * b : 4 * b + 4].reshape(R, D) * 0.25
        ).astype(np.float32)
        ac4 = np.zeros((128, D), dtype=np.float32)
        for j in range(4):
            ac4[32 * j : 32 * j + R, :] = ac
        in_maps.append(
            {
                "xt": xt,
                "bcat4": bc4.astype(NPBF16),
                "acat4": ac4.astype(NPBF16),
            }
        )
    return in_maps


def run(x, adapter_b, adapter_a, **run_kwargs):
    nc = _get_nc()
    in_maps = make_in_maps(x, adapter_b, adapter_a)
    res = run_bass_kernel_spmd(nc, in_maps, list(range(N_CORES)), **run_kwargs)
    # y packed [sl, t, p, d] -> [s, d]: (sl, t, p) is lexicographic in s
    out = np.stack(
        [
            res.results[i]["y"].reshape(S, D).astype(np.float32)
            for i in range(N_CORES)
        ]
    )
    return out, res


def kernel(x, adapter_b, adapter_a):
    out, _ = run(x, adapter_b, adapter_a)
    return out
